# revision 7
# baseline (speedup 1.0000x reference)
"""Trainium2 Bass kernel for LDPC sum-product BP decoding (nn_BP_Decoder).

Takes FULL unsharded inputs (llr_demapper [1024, 2040] plus Tanner-graph
index arrays), data-parallel over the batch axis across 8 NeuronCores
(128 batch rows per core = the SBUF partition count), returns the FULL
[1024, 2040] float32 output.

Math (per core, batch rows on the 128 SBUF partitions):
  The (3,6)-regular Gallager code decomposes into 3 row-blocks; block b's
  edges are a permutation of the 2040 variables (block 0 is the identity).
  check->var per block: exclude-self tanh-product via prefix/suffix chains,
  cv = ln(1+M*x) - ln(1-M*x) with M = 1-1e-7 folded into the ACT affine.
  var->check: v-space sums W_b = llr + sum_{b'!=b} cv_b'; the permuted
  blocks' messages cross via GPSIMD local_scatter in fp16.

This revision vs the first working version:
  - All block/edge spaces use a SLAB-major device layout (slot-major:
    device position = 340*k + g for check-group g, slot k) so every
    prefix/suffix product is a contiguous [128,340] slice.  Contiguous
    fp16 tensor_tensor runs ~4x faster than the strided fp32 original
    (167ns vs 722ns measured), shortening the serial c2v chain that sits
    between scatters.
  - fp16 throughout the iteration (products, sums, ln outputs); fp32 only
    for the final output sum.  Host-side numpy BP simulation of the full
    fp16 pipeline gives rel err ~3e-5 vs the fp32 reference (budget 2e-2).
  - Host pre-permutes llr into each block's device order (no initial
    scatters) and relabels all scatter index vectors for the slab layouts.

The global sign flip of the reference (llr = -llr_demapper, out =
-llr_dec) cancels by oddness of the whole message-passing, so the kernel
runs directly on llr_demapper.
"""
import functools
import numpy as np

import concourse.bacc as bacc
import concourse.tile as tile
import concourse.mybir as mybir
from concourse.tile_rust import add_dep_helper
from contextlib import ExitStack

F32 = mybir.dt.float32
F16 = mybir.dt.float16
I16 = mybir.dt.int16
AF = mybir.ActivationFunctionType
OP = mybir.AluOpType

N = 2040      # variables (and per-block edges)
NGRP = 340    # check groups per block
DC = 6        # check degree
N_CORES = 8
M_CLIP = float(np.float32(1.0) - np.float32(1e-7))


class _Body:
    """BP iteration body on a prebuilt tile set (shared by the unrolled
    kernel and the For_i benchmark)."""

    def __init__(self, nc, tc, pool, llr, llrp1, llrp2, sidx):
        self.nc, self.tc, self.pool = nc, tc, pool
        self.iter_idx = 0

        def t16(tag):
            return pool.tile([128, N], F16, tag=tag, name=tag)

        self.llr_s = t16("llr_s")
        self.llrp1_s = t16("llrp1_s")
        self.llrp2_s = t16("llrp2_s")
        self.idx_s = pool.tile([128, 4 * N], I16, tag="idx_s", name="idx_s")
        nc.sync.dma_start(self.llr_s[:], llr)
        nc.sync.dma_start(self.llrp1_s[:], llrp1)
        nc.sync.dma_start(self.llrp2_s[:], llrp2)
        nc.sync.dma_start(self.idx_s[:], sidx)
        self.ix_inv1 = self.idx_s[:, 0 * N:1 * N]
        self.ix_inv2 = self.idx_s[:, 1 * N:2 * N]
        self.ix_perm1 = self.idx_s[:, 2 * N:3 * N]
        self.ix_perm2 = self.idx_s[:, 3 * N:4 * N]

        self.t_b = [t16(f"t{b}") for b in range(3)]
        self.ps_b = [t16(f"ps{b}") for b in range(3)]
        self.ex_b = [t16(f"ex{b}") for b in range(3)]
        self.W0 = t16("W0")
        self.u = t16("u")
        self.cv0p = t16("cv0p")
        self.a0 = t16("a0")
        self.cvp1h = t16("cvp1h")
        self.cvp2h = t16("cvp2h")
        self.cv1h = t16("cv1h")
        self.cv2h = t16("cv2h")
        self.W1h = t16("W1h")
        self.W2h = t16("W2h")
        self.x1h = t16("x1h")
        self.x2h = t16("x2h")
        self.prev_scat = []

    def c2v_block(self, b, src_ap):
        """tanh + slab-major exclude-products + two lns for one block."""
        nc = self.nc
        t = self.t_b[b]
        th = nc.scalar.activation(t[:], src_ap, AF.Tanh, scale=0.5)
        tm = [t[:, NGRP * k:NGRP * (k + 1)] for k in range(DC)]
        pre = [self.ps_b[b][:, NGRP * j:NGRP * (j + 1)] for j in range(3)]
        suf = [self.ps_b[b][:, NGRP * (3 + j):NGRP * (4 + j)] for j in range(3)]
        ex = [self.ex_b[b][:, NGRP * k:NGRP * (k + 1)] for k in range(DC)]
        v = nc.vector
        v.tensor_tensor(pre[0], tm[0], tm[1], OP.mult)
        v.tensor_tensor(pre[1], pre[0], tm[2], OP.mult)
        v.tensor_tensor(pre[2], pre[1], tm[3], OP.mult)
        v.tensor_tensor(ex[5], pre[2], tm[4], OP.mult)
        v.tensor_tensor(suf[0], tm[5], tm[4], OP.mult)
        v.tensor_tensor(suf[1], suf[0], tm[3], OP.mult)
        v.tensor_tensor(suf[2], suf[1], tm[2], OP.mult)
        v.tensor_tensor(ex[0], suf[2], tm[1], OP.mult)
        v.tensor_tensor(ex[1], tm[0], suf[2], OP.mult)
        v.tensor_tensor(ex[2], pre[0], suf[1], OP.mult)
        v.tensor_tensor(ex[3], pre[1], suf[0], OP.mult)
        v.tensor_tensor(ex[4], pre[2], tm[5], OP.mult)
        lnA = nc.scalar.activation(self.t_b[b][:], self.ex_b[b][:], AF.Ln,
                                   scale=M_CLIP, bias=1.0)
        lnB = nc.scalar.activation(self.ps_b[b][:], self.ex_b[b][:], AF.Ln,
                                   scale=-M_CLIP, bias=1.0)
        return th, lnA, lnB

    def prologue(self):
        """Initial c2v on the host-permuted llr blocks + cv subs."""
        nc = self.nc
        r0 = self.c2v_block(0, self.llr_s[:])
        r1 = self.c2v_block(1, self.llrp1_s[:])
        r2 = self.c2v_block(2, self.llrp2_s[:])
        for ln in (r0[1], r0[2], r1[1], r1[2], r2[1], r2[2]):
            for th in (r0[0], r1[0], r2[0]):
                add_dep_helper(ln.ins, th.ins, sync=False,
                               reason="tanh before ln (ACT table set)")
        nc.vector.tensor_tensor(self.cvp1h[:], self.t_b[1][:],
                                self.ps_b[1][:], OP.subtract)
        nc.vector.tensor_tensor(self.cvp2h[:], self.t_b[2][:],
                                self.ps_b[2][:], OP.subtract)
        nc.vector.tensor_tensor(self.cv0p[:], self.t_b[0][:],
                                self.ps_b[0][:], OP.subtract)

    def scat(self, dst, src, ix):
        return self.nc.gpsimd.local_scatter(dst[:], src[:], ix, channels=128,
                                            num_elems=N, num_idxs=N)

    def iteration(self):
        nc = self.nc
        s1 = self.scat(self.cv1h, self.cvp1h, self.ix_perm1)
        nc.vector.tensor_tensor(self.a0[:], self.llr_s[:], self.cv0p[:],
                                OP.add)
        s2 = self.scat(self.cv2h, self.cvp2h, self.ix_perm2)
        nc.vector.tensor_tensor(self.W2h[:], self.a0[:], self.cv1h[:], OP.add)
        nc.vector.tensor_tensor(self.u[:], self.cv1h[:], self.llr_s[:], OP.add)
        nc.vector.tensor_tensor(self.W1h[:], self.a0[:], self.cv2h[:], OP.add)
        s3 = self.scat(self.x1h, self.W1h, self.ix_inv1)
        nc.vector.tensor_tensor(self.W0[:], self.u[:], self.cv2h[:], OP.add)
        s4 = self.scat(self.x2h, self.W2h, self.ix_inv2)
        scats = (self.prev_scat[-1:] if self.iter_idx else []) + [s1, s2, s3, s4]
        for a, b in zip(scats[1:], scats):
            add_dep_helper(a.ins, b.ins, sync=False, reason="pool order")
        self.prev_scat = [s4]
        self.iter_idx += 1
        # relaxed ACT grouping: [tanh0 tanh1][ln0 ln1][tanh2][ln2]
        r0 = self.c2v_block(0, self.W0[:])
        r1 = self.c2v_block(1, self.x1h[:])
        r2 = self.c2v_block(2, self.x2h[:])
        for ln in (r0[1], r0[2], r1[1], r1[2]):
            for th in (r0[0], r1[0]):
                add_dep_helper(ln.ins, th.ins, sync=False, reason="act-order")
            add_dep_helper(r2[0].ins, ln.ins, sync=False, reason="act-order")
        nc.vector.tensor_tensor(self.cvp1h[:], self.t_b[1][:],
                                self.ps_b[1][:], OP.subtract)
        nc.vector.tensor_tensor(self.cv0p[:], self.t_b[0][:],
                                self.ps_b[0][:], OP.subtract)
        nc.vector.tensor_tensor(self.cvp2h[:], self.t_b[2][:],
                                self.ps_b[2][:], OP.subtract)

    def epilogue(self, out):
        nc = self.nc
        s1 = self.scat(self.cv1h, self.cvp1h, self.ix_perm1)
        nc.vector.tensor_tensor(self.a0[:], self.llr_s[:], self.cv0p[:],
                                OP.add)
        s2 = self.scat(self.cv2h, self.cvp2h, self.ix_perm2)
        for a, b in zip([s1, s2], self.prev_scat + [s1]):
            add_dep_helper(a.ins, b.ins, sync=False, reason="pool order")
        nc.vector.tensor_tensor(self.u[:], self.a0[:], self.cv1h[:], OP.add)
        S32 = self.pool.tile([128, N], F32, tag="S32", name="S32")
        nc.vector.tensor_tensor(S32[:], self.u[:], self.cv2h[:], OP.add)
        nc.sync.dma_start(out, S32[:])


def make_body(nc, tc, pool, llr, llrp1, llrp2, sidx):
    return _Body(nc, tc, pool, llr, llrp1, llrp2, sidx)


@functools.lru_cache(maxsize=2)
def _build_bp(nb_iter):
    nc = bacc.Bacc("TRN2", target_bir_lowering=False, debug=False,
                   enable_asserts=False, num_devices=N_CORES)
    llr = nc.dram_tensor("llr", [128, N], F16, kind="ExternalInput").ap()
    llrp1 = nc.dram_tensor("llrp1", [128, N], F16, kind="ExternalInput").ap()
    llrp2 = nc.dram_tensor("llrp2", [128, N], F16, kind="ExternalInput").ap()
    sidx = nc.dram_tensor("sidx", [128, 4 * N], I16, kind="ExternalInput").ap()
    out = nc.dram_tensor("out", [128, N], F32, kind="ExternalOutput").ap()

    with tile.TileContext(nc) as tc, ExitStack() as ctx:
        pool = ctx.enter_context(tc.tile_pool(name="p", bufs=1))
        body = _Body(nc, tc, pool, llr, llrp1, llrp2, sidx)
        body.prologue()
        for _ in range(nb_iter):
            body.iteration()
        body.epilogue(out)
    nc.compile()
    return nc


# --- host-side layout / index preparation ---------------------------------

def _slab(pos):
    """group-major edge position -> slab-major device position."""
    g, k = pos // DC, pos % DC
    return k * NGRP + g


@functools.lru_cache(maxsize=1)
def _prep_graph(vn_msg_key):
    vg = np.frombuffer(vn_msg_key, dtype=np.int64).reshape(N, 3)
    inv1 = vg[:, 1] - N          # b1 edge position of var v (group-major)
    inv2 = vg[:, 2] - 2 * N
    perm1 = np.argsort(inv1)     # var at b1 edge position j
    perm2 = np.argsort(inv2)

    pos = np.arange(N)
    sl = _slab(pos)              # group-major -> slab-major
    border = np.empty(N, np.int64)   # slab-major device pos -> group-major
    border[sl] = pos

    vpos = sl                    # var v -> v-space device position (b0 slab)
    vorder = border              # device position -> var (b0: var == position)
    bpos1 = sl                   # b1 edge j -> b1 device position
    bpos2 = sl

    # s1: dst[ix1[p]] = cvp1h[p]; data pos p holds b1-edge border[p],
    # whose variable lands at v-device-pos vpos[perm1[border[p]]].
    ix1 = vpos[perm1[border]]
    ix2 = vpos[perm2[border]]
    # s3: data pos p holds var vorder[p]; dest b1 device pos of its b1 edge.
    ix3 = bpos1[inv1[vorder]]
    ix4 = bpos2[inv2[vorder]]
    sidx = np.concatenate([ix3, ix4, ix1, ix2]).astype(np.int16)
    return (perm1, perm2, vpos, vorder, border,
            np.ascontiguousarray(np.tile(sidx[None, :], (128, 1))))


def _host_inputs(llr, vn_msg_ind):
    (perm1, perm2, vpos, vorder, border, sidx) = _prep_graph(
        np.asarray(vn_msg_ind, dtype=np.int64).tobytes())
    # device llr layouts: v-slab order and per-block slab orders
    lv = np.ascontiguousarray(llr[:, vorder]).astype(np.float16)
    l1 = np.ascontiguousarray(llr[:, perm1[border]]).astype(np.float16)
    l2 = np.ascontiguousarray(llr[:, perm2[border]]).astype(np.float16)
    return lv, l1, l2, sidx, vorder


class _Runner:
    """jit-compiled PJRT executor for a prebuilt Bass module on 8 cores."""

    def __init__(self, nc):
        import jax
        from jax.sharding import Mesh, PartitionSpec
        from jax.experimental.shard_map import shard_map
        from concourse.bass2jax import (_bass_exec_p, install_neuronx_cc_hook,
                                        partition_id_tensor)
        install_neuronx_cc_hook()
        self.jax = jax
        partition_name = (nc.partition_id_tensor.name
                          if nc.partition_id_tensor else None)
        in_names, out_names, out_avals, zero_outs = [], [], [], []
        for alloc in nc.m.functions[0].allocations:
            if not isinstance(alloc, mybir.MemoryLocationSet):
                continue
            name = alloc.memorylocations[0].name
            if alloc.kind == "ExternalInput":
                if name != partition_name:
                    in_names.append(name)
            elif alloc.kind == "ExternalOutput":
                out_names.append(name)
                shape = tuple(alloc.tensor_shape)
                dtype = mybir.dt.np(alloc.dtype)
                out_avals.append(jax.core.ShapedArray(shape, dtype))
                zero_outs.append(np.zeros(shape, dtype))
        self.in_names, self.out_names = in_names, out_names
        self.out_avals, self.zero_outs = out_avals, zero_outs
        n_params, n_outs = len(in_names), len(out_avals)
        all_in = tuple(in_names + out_names
                       + ([partition_name] if partition_name else []))
        donate = tuple(range(n_params, n_params + n_outs))

        def _body(*args):
            operands = list(args)
            if partition_name is not None:
                operands.append(partition_id_tensor())
            return tuple(_bass_exec_p.bind(
                *operands, out_avals=tuple(out_avals), in_names=all_in,
                out_names=tuple(out_names), lowering_input_output_aliases=(),
                sim_require_finite=True, sim_require_nnan=True, nc=nc))

        devices = jax.devices()[:N_CORES]
        mesh = Mesh(np.asarray(devices), ("core",))
        self.fn = jax.jit(
            shard_map(_body, mesh=mesh,
                      in_specs=(PartitionSpec("core"),) * (n_params + n_outs),
                      out_specs=(PartitionSpec("core"),) * n_outs,
                      check_rep=False),
            donate_argnums=donate, keep_unused=True)

    def run(self, in_maps):
        per_core = [[np.asarray(m[n]) for n in self.in_names] for m in in_maps]
        args = [np.concatenate([per_core[c][i] for c in range(N_CORES)], axis=0)
                for i in range(len(self.in_names))]
        args += [np.zeros((N_CORES * z.shape[0], *z.shape[1:]), z.dtype)
                 for z in self.zero_outs]
        outs = self.fn(*[self.jax.numpy.asarray(a) for a in args])
        self.jax.block_until_ready(outs)
        return [{n: np.asarray(outs[i]).reshape(N_CORES, *self.out_avals[i].shape)[c]
                 for i, n in enumerate(self.out_names)} for c in range(N_CORES)]


_runner_cache = {}


def _get_runner(nb_iter):
    if nb_iter not in _runner_cache:
        _runner_cache[nb_iter] = _Runner(_build_bp(nb_iter))
    return _runner_cache[nb_iter]


def kernel(llr_demapper, cn_msg_ind, vn_msg_ind, vn2cn_ind, cn_mask_ind,
           vn_mask_ind, edge_vn, nb_iter):
    llr = np.asarray(llr_demapper, dtype=np.float32)
    B = llr.shape[0]
    assert llr.shape == (B, N) and B % N_CORES == 0
    nb_iter = int(np.asarray(nb_iter))

    lv, l1, l2, sidx, vorder = _host_inputs(llr, vn_msg_ind)

    rows = B // N_CORES
    assert rows == 128, "kernel is specialized for 128 batch rows per core"
    in_maps = []
    for c in range(N_CORES):
        sl = slice(c * rows, (c + 1) * rows)
        in_maps.append({
            "llr": lv[sl],
            "llrp1": l1[sl],
            "llrp2": l2[sl],
            "sidx": sidx,
        })

    runner = _get_runner(nb_iter)
    res = runner.run(in_maps)
    dev_out = np.concatenate([r["out"] for r in res], axis=0)
    out = np.empty_like(dev_out)
    out[:, vorder] = dev_out          # undo the v-space slab layout
    return out


def make_bench_inputs():
    """Synthetic single-core inputs for bench_bp.py."""
    rng = np.random.default_rng(0)
    llr = rng.standard_normal((128, N)).astype(np.float32)
    # synthetic permutations; only the structure matters for timing
    p1 = rng.permutation(N)
    p2 = rng.permutation(N)
    vg = np.stack([np.arange(N), np.argsort(p1) + N,
                   np.argsort(p2) + 2 * N], axis=1)
    lv, l1, l2, sidx, vorder = _host_inputs(llr, vg.reshape(-1))
    return {"llr": lv, "llrp1": l1, "llrp2": l2, "sidx": sidx}
